# revision 13
# baseline (speedup 1.0000x reference)
"""InfoNCE (CPIC) loss kernel for Trainium2, 8 NeuronCores.

Math (B=1024, D=256):
  scores[i,j] = -0.5 * sum_d( log vc[j,d] + (y[i,d]-m[j,d])^2 / vc[j,d] )
    where vc = where(v < 1e-6, v + 1e-6, v)
  mi_lower = log(B) + mean_i(diag_i - logsumexp_j scores[i,:])
  mi_upper = mean_i(diag_i - (logsumexp_{j!=i} scores[i,:] - log(B-1)))
  out = [mi_lower, mi_upper]

Decomposition on device (per core c, rows i in [128c, 128c+128)):
  raw[i,j] = sum_d y2[i,d]*r[j,d] + sum_d y[i,d]*u2[j,d] + a[j]
    r  = 1/vc, u2 = -2*m*r, a[j] = sum_d (log vc + m^2 r)
  scores = -0.5*raw.  All contractions run on the PE (K=512 accumulation
  + ones-weight matmuls that broadcast-add a[j] into every row + an
  identity-weight matmul that adds the host diag mask * 2^60).
  Per-row: extract diag (mask multiply + row-sum), add 2^60 at the diag
  (PE), min_j raw (= row max of scores, diag excluded), then
  e = exp(-0.5*raw - max) with fused per-partition sum (accum_out).
Device output per core: [128, 4] = (diag_raw*2^60, min_raw, sum_e, 0).
Host: logs/means (the cheap cross-shard reduction).

Sharding: row-shard of y; x_mean/x_vars broadcast to all 8 cores.  Host
passes transposed ([D, B]) views so no on-device transposes are needed.
"""

import os
import sys

import numpy as np

sys.path.insert(0, "/opt/trn_rl_repo")

import concourse.bass as bass  # noqa: E402,F401
import concourse.bacc as bacc  # noqa: E402
import concourse.tile as tile  # noqa: E402
from concourse.tile import add_dep_helper  # noqa: E402
import concourse.hw_specs as hw_specs  # noqa: E402
from concourse import mybir  # noqa: E402
from concourse import bass_utils  # noqa: E402
from concourse.dve_ops import (  # noqa: E402
    RECIP_APPROX_FAST_CONSTS,
    RECIPROCAL_APPROX_FAST,
)
from contextlib import ExitStack  # noqa: E402

B = 1024
D = 256
NCORES = 8
ROWS = B // NCORES  # 128
THRESHOLD = 1e-6
BIG = float(2.0**60)

F32 = mybir.dt.float32
F32R = mybir.dt.float32r
AX = mybir.AxisListType
OP = mybir.AluOpType
AF = mybir.ActivationFunctionType

# matmul operand dtype: float32r streams at 1 col/cycle (4x faster than
# float32, ~2^-13 mantissa rounding); flip env var for exact-fp32 matmuls.
MM_F32R = os.environ.get("KERNEL_MM_DTYPE", "f32r") == "f32r"
MDT = F32R if MM_F32R else F32
RECIP = os.environ.get("KERNEL_RECIP", "fast")  # fast | exact

_ACT_SET = "natural_log_exp_and_others"


def _patch_act_tables():
    """Make every activation resolve to the one set that holds ln+exp+
    square+copy, so only one ACT_TABLE_LOAD (~1.3us) is emitted.  Other
    set entries are emptied, not removed (act_func_set_id is positional)."""
    if getattr(hw_specs, "_ant_act_patch", None):
        return
    orig = hw_specs.get_activation_tables

    def patched(arch):
        tabs = orig(arch)
        if _ACT_SET not in tabs:
            return tabs
        return {k: (v if k == _ACT_SET else set()) for k, v in tabs.items()}

    hw_specs._ant_act_patch = True
    hw_specs.get_activation_tables = patched
    for mod in (bacc, bass):
        if hasattr(mod, "get_activation_tables"):
            mod.get_activation_tables = patched


def _recip(nc, out_ap, in_ap):
    if RECIP == "exact":
        nc.vector.reciprocal(out_ap, in_ap)
    else:
        c = RECIP_APPROX_FAST_CONSTS
        nc.vector._custom_dve(
            RECIPROCAL_APPROX_FAST, out=out_ap, in0=in_ap,
            s0=c["s0"], s1=c["s1"], imm2=c["imm2"],
        )


def _build():
    _patch_act_tables()
    nc = bacc.Bacc("TRN2", target_bir_lowering=False, debug=False, num_devices=8)
    mT = nc.declare_dram_parameter("mT", [D, B], F32, isOutput=False)
    vT = nc.declare_dram_parameter("vT", [D, B], F32, isOutput=False)
    yT = nc.declare_dram_parameter("yT", [D, ROWS], MDT, isOutput=False)
    consts = nc.declare_dram_parameter("consts", [128, B + 256], MDT, isOutput=False)
    out = nc.declare_dram_parameter("out", [ROWS, 8], F32, isOutput=True)

    KC = D // 128  # 2 contraction chunks

    with ExitStack() as ctx:
        tc = ctx.enter_context(tile.TileContext(nc))
        pool = ctx.enter_context(tc.tile_pool(name="main", bufs=1))
        ppool = ctx.enter_context(tc.tile_pool(name="psum", bufs=1, space="PSUM"))

        v_t = pool.tile([128, KC * B], F32, name="v")
        m_t = pool.tile([128, KC * B], F32, name="m")
        y_t = pool.tile([128, KC * ROWS], MDT, name="y")
        y2_t = pool.tile([128, KC * ROWS], MDT, name="y2")
        vc_t = [pool.tile([128, B], F32, name=f"vc{k}") for k in range(KC)]
        tm_t = [pool.tile([128, B], F32, name=f"tm{k}") for k in range(KC)]
        r_t = [pool.tile([128, B], MDT, name=f"r{k}") for k in range(KC)]
        u2_t = [pool.tile([128, B], MDT, name=f"u2{k}") for k in range(KC)]
        mu_t = [pool.tile([128, B], MDT, name=f"mu{k}") for k in range(KC)]
        lv_t = [pool.tile([128, B], MDT, name=f"lv{k}") for k in range(KC)]
        consts_t = pool.tile([128, B + 256], MDT, name="consts")
        msk_t = consts_t[:, 0:B]
        iden_t = consts_t[:, B:B + 128]
        ones_t = consts_t[:, B + 128:B + 256]
        scr_t = pool.tile([ROWS, B], F32, name="scr")
        e_t = pool.tile([ROWS, B], F32, name="e")
        o_t = pool.tile([ROWS, 8], F32, name="o")
        bias2_t = pool.tile([ROWS, 2], F32, name="bias2")
        bias_t = pool.tile([ROWS, 1], F32, name="bias")

        psum_s = ppool.tile([ROWS, B], F32, name="scores")

        vT3 = vT.rearrange("(c p) b -> p c b", p=128)
        mT3 = mT.rearrange("(c p) b -> p c b", p=128)
        yT3 = yT.rearrange("(c p) i -> p c i", p=128)

        nc.gpsimd.memset(o_t[:], 0.0)
        nc.gpsimd.dma_start(out=v_t[:, 0:B], in_=vT3[:, 0, :])
        nc.scalar.dma_start(out=m_t[:, 0:B], in_=mT3[:, 0, :])
        nc.sync.dma_start(out=v_t[:, B:], in_=vT3[:, 1, :])
        nc.scalar.dma_start(out=m_t[:, B:], in_=mT3[:, 1, :])
        nc.sync.dma_start(out=y_t[:].rearrange("p (c i) -> p c i", c=KC), in_=yT3)
        nc.scalar.dma_start(out=consts_t[:], in_=consts[:, :])

        prev_last = None
        with nc.allow_low_precision(reason="f32r matmul operands"):
            nc.scalar.activation(y2_t[:], y_t[:], AF.Square)
            for k in range(KC):
                vk = v_t[:, k * B:(k + 1) * B]
                mk = m_t[:, k * B:(k + 1) * B]
                # vc = v + T*(v < T)
                i_ts = nc.vector.tensor_scalar(
                    out=tm_t[k][:], in0=vk, scalar1=float(THRESHOLD),
                    scalar2=float(THRESHOLD), op0=OP.is_lt, op1=OP.mult,
                )
                if prev_last is not None:
                    # keep the DVE stream chunk-major: chunk k+1 must not
                    # sit ahead of chunk k's chain (head-of-line on DMA wait)
                    add_dep_helper(i_ts.ins, prev_last.ins, sync=False,
                                   reason="chunk order")
                nc.vector.tensor_add(vc_t[k][:], tm_t[k][:], vk)
                _recip(nc, r_t[k][:], vc_t[k][:])
                # u2 = -2*m*r ; mu = m^2*r = (m*-0.5)*u2
                nc.vector.scalar_tensor_tensor(
                    out=u2_t[k][:], in0=mk, scalar=-2.0, in1=r_t[k][:],
                    op0=OP.mult, op1=OP.mult,
                )
                prev_last = nc.vector.scalar_tensor_tensor(
                    out=mu_t[k][:], in0=mk, scalar=-0.5, in1=u2_t[k][:],
                    op0=OP.mult, op1=OP.mult,
                )
                nc.scalar.activation(lv_t[k][:], vc_t[k][:], AF.Ln)

        # raw = y2.T@r + y.T@u2 + ones.T@(lv+mu)   (per 512-col PSUM bank),
        # then per-bank: diag partial (mask multiply+reduce), diag mask add
        # on the PE (I.T @ msk, msk holds 2^60 at diag), partial row min.
        dtmp = pool.tile([ROWS, 2], F32, name="dtmp")
        mtmp = pool.tile([ROWS, 2], F32, name="mtmp")
        NB = B // 512
        for nb in range(NB):
            nsl = slice(nb * 512, (nb + 1) * 512)
            seq = []
            for k in range(KC):
                ksl = slice(k * ROWS, (k + 1) * ROWS)
                seq.append((y2_t[:, ksl], r_t[k][:, nsl]))
                seq.append((y_t[:, ksl], u2_t[k][:, nsl]))
            for k in range(KC):
                seq.append((ones_t[:], lv_t[k][:, nsl]))
                seq.append((ones_t[:], mu_t[k][:, nsl]))
            for si, (lhsT, rhs) in enumerate(seq):
                nc.tensor.matmul(
                    psum_s[:, nsl], lhsT, rhs,
                    start=(si == 0), stop=(si == len(seq) - 1),
                )
        for nb in range(NB):
            nsl = slice(nb * 512, (nb + 1) * 512)
            nc.vector.tensor_mul(scr_t[:, nsl], psum_s[:, nsl], msk_t[:, nsl])
            nc.vector.tensor_reduce(
                out=o_t[:, nb:nb + 1], in_=scr_t[:, nsl], axis=AX.X, op=OP.add,
            )
            nc.tensor.matmul(
                psum_s[:, nsl], iden_t[:], msk_t[:, nsl],
                start=False, stop=True, skip_group_check=True,
            )
            nc.vector.tensor_reduce(
                out=o_t[:, 2 + nb:3 + nb], in_=psum_s[:, nsl], axis=AX.X, op=OP.min,
            )
            # per-bank e = exp(-0.5*raw + 0.5*min_b); S_b = sum_j e (fused);
            # banks are merged on the host like shards
            nc.vector.tensor_scalar_mul(
                bias2_t[:, nb:nb + 1], o_t[:, 2 + nb:3 + nb], 0.5)
            nc.scalar.activation(
                e_t[:, nsl], psum_s[:, nsl], AF.Exp,
                bias=bias2_t[:, nb:nb + 1], scale=-0.5,
                accum_out=o_t[:, 4 + nb:5 + nb],
            )

        nc.sync.dma_start(out=out[:, :], in_=o_t[:])

    nc.finalize()
    return nc


_CACHE = {}


def _get_nc():
    if "nc" not in _CACHE:
        _CACHE["nc"] = _build()
    return _CACHE["nc"]


def _in_maps(x_mean, x_vars, y):
    m = np.ascontiguousarray(np.asarray(x_mean, dtype=np.float32))
    v = np.ascontiguousarray(np.asarray(x_vars, dtype=np.float32))
    yv = np.ascontiguousarray(np.asarray(y, dtype=np.float32))
    mT = np.ascontiguousarray(m.T)
    vT = np.ascontiguousarray(v.T)
    p = np.arange(ROWS)
    maps = []
    for c in range(NCORES):
        yTs = np.ascontiguousarray(yv[c * ROWS:(c + 1) * ROWS].T)
        consts = np.zeros((128, B + 256), np.float32)
        consts[p, c * ROWS + p] = np.float32(BIG)          # msk
        consts[p, B + p] = 1.0                             # iden
        consts[:, B + 128:B + 256] = 1.0                   # ones
        maps.append({"mT": mT, "vT": vT, "yT": yTs, "consts": consts})
    return maps


def _combine(results):
    outs = np.concatenate([results[c]["out"] for c in range(NCORES)], axis=0)
    o = outs.astype(np.float64)
    diag = -0.5 * (o[:, 0] + o[:, 1]) / BIG
    lse0 = -0.5 * o[:, 2] + np.log(o[:, 4])
    lse1 = -0.5 * o[:, 3] + np.log(o[:, 5])
    lse_nd = np.logaddexp(lse0, lse1)
    lse_f = np.logaddexp(lse_nd, diag)
    mi_lower = np.log(float(B)) + np.mean(diag - lse_f)
    mi_upper = np.mean(diag - lse_nd) + np.log(float(B - 1))
    return np.array([mi_lower, mi_upper], dtype=np.float32)


def _run(x_mean, x_vars, y, **kw):
    nc = _get_nc()
    res = bass_utils.run_bass_kernel_spmd(
        nc, _in_maps(x_mean, x_vars, y), list(range(NCORES)), **kw
    )
    return _combine(res.results), res


def kernel(x_mean, x_vars, y):
    return _run(x_mean, x_vars, y)[0]


# revision 14
# speedup vs baseline: 1.1398x; 1.1398x over previous
"""InfoNCE (CPIC) loss kernel for Trainium2, 8 NeuronCores.

Math (B=1024, D=256):
  scores[i,j] = -0.5 * sum_d( log vc[j,d] + (y[i,d]-m[j,d])^2 / vc[j,d] )
    where vc = where(v < 1e-6, v + 1e-6, v)
  mi_lower = log(B) + mean_i(diag_i - logsumexp_j scores[i,:])
  mi_upper = mean_i(diag_i - (logsumexp_{j!=i} scores[i,:] - log(B-1)))
  out = [mi_lower, mi_upper]

Decomposition on device (per core c, rows i in [128c, 128c+128)):
  raw[i,j] = sum_d y2[i,d]*r[j,d] + sum_d y[i,d]*u2[j,d] + a[j]
    r  = 1/vc, u2 = -2*m*r, a[j] = sum_d (log vc + m^2 r)
  scores = -0.5*raw.  All contractions run on the PE (K=512 accumulation
  + ones-weight matmuls that broadcast-add a[j] into every row + an
  identity-weight matmul that adds the host diag mask * 2^60).
  Per-row: extract diag (mask multiply + row-sum), add 2^60 at the diag
  (PE), min_j raw (= row max of scores, diag excluded), then
  e = exp(-0.5*raw - max) with fused per-partition sum (accum_out).
Device output per core: [128, 4] = (diag_raw*2^60, min_raw, sum_e, 0).
Host: logs/means (the cheap cross-shard reduction).

Sharding: row-shard of y; x_mean/x_vars broadcast to all 8 cores.  Host
passes transposed ([D, B]) views so no on-device transposes are needed.
"""

import os
import sys

import numpy as np

sys.path.insert(0, "/opt/trn_rl_repo")

import concourse.bass as bass  # noqa: E402,F401
import concourse.bacc as bacc  # noqa: E402
import concourse.tile as tile  # noqa: E402
from concourse.tile import add_dep_helper  # noqa: E402
import concourse.hw_specs as hw_specs  # noqa: E402
from concourse import mybir  # noqa: E402
from concourse import bass_utils  # noqa: E402
from concourse.dve_ops import (  # noqa: E402
    RECIP_APPROX_FAST_CONSTS,
    RECIPROCAL_APPROX_FAST,
)
from contextlib import ExitStack  # noqa: E402

B = 1024
D = 256
NCORES = 8
ROWS = B // NCORES  # 128
THRESHOLD = 1e-6
BIG = float(2.0**60)

F32 = mybir.dt.float32
F32R = mybir.dt.float32r
AX = mybir.AxisListType
OP = mybir.AluOpType
AF = mybir.ActivationFunctionType

# matmul operand dtype: float32r streams at 1 col/cycle (4x faster than
# float32, ~2^-13 mantissa rounding); flip env var for exact-fp32 matmuls.
MM_F32R = os.environ.get("KERNEL_MM_DTYPE", "f32r") == "f32r"
MDT = F32R if MM_F32R else F32
RECIP = os.environ.get("KERNEL_RECIP", "fast")  # fast | exact

_ACT_SET = "natural_log_exp_and_others"


def _patch_act_tables():
    """Make every activation resolve to the one set that holds ln+exp+
    square+copy, so only one ACT_TABLE_LOAD (~1.3us) is emitted.  Other
    set entries are emptied, not removed (act_func_set_id is positional)."""
    if getattr(hw_specs, "_ant_act_patch", None):
        return
    orig = hw_specs.get_activation_tables

    def patched(arch):
        tabs = orig(arch)
        if _ACT_SET not in tabs:
            return tabs
        return {k: (v if k == _ACT_SET else set()) for k, v in tabs.items()}

    hw_specs._ant_act_patch = True
    hw_specs.get_activation_tables = patched
    for mod in (bacc, bass):
        if hasattr(mod, "get_activation_tables"):
            mod.get_activation_tables = patched


def _recip(nc, out_ap, in_ap):
    if RECIP == "exact":
        nc.vector.reciprocal(out_ap, in_ap)
    else:
        c = RECIP_APPROX_FAST_CONSTS
        nc.vector._custom_dve(
            RECIPROCAL_APPROX_FAST, out=out_ap, in0=in_ap,
            s0=c["s0"], s1=c["s1"], imm2=c["imm2"],
        )


def _build():
    _patch_act_tables()
    nc = bacc.Bacc("TRN2", target_bir_lowering=False, debug=False, num_devices=8)
    mT = nc.declare_dram_parameter("mT", [D, B], F32, isOutput=False)
    vT = nc.declare_dram_parameter("vT", [D, B], F32, isOutput=False)
    yT = nc.declare_dram_parameter("yT", [D, ROWS], MDT, isOutput=False)
    consts = nc.declare_dram_parameter("consts", [128, B + 256], MDT, isOutput=False)
    out = nc.declare_dram_parameter("out", [ROWS, 8], F32, isOutput=True)

    KC = D // 128  # 2 contraction chunks

    with ExitStack() as ctx:
        tc = ctx.enter_context(tile.TileContext(nc))
        pool = ctx.enter_context(tc.tile_pool(name="main", bufs=1))
        ppool = ctx.enter_context(tc.tile_pool(name="psum", bufs=1, space="PSUM"))

        v_t = pool.tile([128, KC * B], F32, name="v")
        m_t = pool.tile([128, KC * B], F32, name="m")
        y_t = pool.tile([128, KC * ROWS], MDT, name="y")
        y2_t = pool.tile([128, KC * ROWS], MDT, name="y2")
        vc_t = [pool.tile([128, B], F32, name=f"vc{k}") for k in range(KC)]
        tm_t = [pool.tile([128, B], F32, name=f"tm{k}") for k in range(KC)]
        r_t = [pool.tile([128, B], MDT, name=f"r{k}") for k in range(KC)]
        u2_t = [pool.tile([128, B], MDT, name=f"u2{k}") for k in range(KC)]
        mu_t = [pool.tile([128, B], MDT, name=f"mu{k}") for k in range(KC)]
        lv_t = [pool.tile([128, B], MDT, name=f"lv{k}") for k in range(KC)]
        consts_t = pool.tile([128, B + 256], MDT, name="consts")
        msk_t = consts_t[:, 0:B]
        iden_t = consts_t[:, B:B + 128]
        ones_t = consts_t[:, B + 128:B + 256]
        scr_t = pool.tile([ROWS, B], F32, name="scr")
        e_t = pool.tile([ROWS, B], F32, name="e")
        o_t = pool.tile([ROWS, 8], F32, name="o")
        bias2_t = pool.tile([ROWS, 2], F32, name="bias2")
        bias_t = pool.tile([ROWS, 1], F32, name="bias")

        psum_s = ppool.tile([ROWS, B], F32, name="scores")

        vT3 = vT.rearrange("(c p) b -> p c b", p=128)
        mT3 = mT.rearrange("(c p) b -> p c b", p=128)
        yT3 = yT.rearrange("(c p) i -> p c i", p=128)

        nc.gpsimd.memset(o_t[:], 0.0)
        nc.sync.dma_start(out=v_t[:, 0:B], in_=vT3[:, 0, :])
        nc.scalar.dma_start(out=m_t[:, 0:B], in_=mT3[:, 0, :])
        nc.sync.dma_start(out=v_t[:, B:], in_=vT3[:, 1, :])
        nc.scalar.dma_start(out=m_t[:, B:], in_=mT3[:, 1, :])
        nc.sync.dma_start(out=y_t[:].rearrange("p (c i) -> p c i", c=KC), in_=yT3)
        nc.scalar.dma_start(out=consts_t[:], in_=consts[:, :])

        prev_last = None
        with nc.allow_low_precision(reason="f32r matmul operands"):
            nc.scalar.activation(y2_t[:], y_t[:], AF.Square)
            for k in range(KC):
                vk = v_t[:, k * B:(k + 1) * B]
                mk = m_t[:, k * B:(k + 1) * B]
                # vc = v + T*(v < T)
                i_ts = nc.vector.tensor_scalar(
                    out=tm_t[k][:], in0=vk, scalar1=float(THRESHOLD),
                    scalar2=float(THRESHOLD), op0=OP.is_lt, op1=OP.mult,
                )
                if prev_last is not None:
                    # keep the DVE stream chunk-major: chunk k+1 must not
                    # sit ahead of chunk k's chain (head-of-line on DMA wait)
                    add_dep_helper(i_ts.ins, prev_last.ins, sync=False,
                                   reason="chunk order")
                nc.vector.tensor_add(vc_t[k][:], tm_t[k][:], vk)
                _recip(nc, r_t[k][:], vc_t[k][:])
                # u2 = -2*m*r ; mu = m^2*r = (m*-0.5)*u2
                nc.vector.scalar_tensor_tensor(
                    out=u2_t[k][:], in0=mk, scalar=-2.0, in1=r_t[k][:],
                    op0=OP.mult, op1=OP.mult,
                )
                prev_last = nc.vector.scalar_tensor_tensor(
                    out=mu_t[k][:], in0=mk, scalar=-0.5, in1=u2_t[k][:],
                    op0=OP.mult, op1=OP.mult,
                )
                nc.scalar.activation(lv_t[k][:], vc_t[k][:], AF.Ln)

        # raw = y2.T@r + y.T@u2 + ones.T@(lv+mu)   (per 512-col PSUM bank),
        # then per-bank: diag partial (mask multiply+reduce), diag mask add
        # on the PE (I.T @ msk, msk holds 2^60 at diag), partial row min.
        dtmp = pool.tile([ROWS, 2], F32, name="dtmp")
        mtmp = pool.tile([ROWS, 2], F32, name="mtmp")
        NB = B // 512
        for nb in range(NB):
            nsl = slice(nb * 512, (nb + 1) * 512)
            seq = []
            for k in range(KC):
                ksl = slice(k * ROWS, (k + 1) * ROWS)
                seq.append((y2_t[:, ksl], r_t[k][:, nsl]))
                seq.append((y_t[:, ksl], u2_t[k][:, nsl]))
            for k in range(KC):
                seq.append((ones_t[:], lv_t[k][:, nsl]))
                seq.append((ones_t[:], mu_t[k][:, nsl]))
            for si, (lhsT, rhs) in enumerate(seq):
                nc.tensor.matmul(
                    psum_s[:, nsl], lhsT, rhs,
                    start=(si == 0), stop=(si == len(seq) - 1),
                )
        for nb in range(NB):
            nsl = slice(nb * 512, (nb + 1) * 512)
            nc.vector.tensor_mul(scr_t[:, nsl], psum_s[:, nsl], msk_t[:, nsl])
            nc.vector.tensor_reduce(
                out=o_t[:, nb:nb + 1], in_=scr_t[:, nsl], axis=AX.X, op=OP.add,
            )
            nc.tensor.matmul(
                psum_s[:, nsl], iden_t[:], msk_t[:, nsl],
                start=False, stop=True, skip_group_check=True,
            )
            nc.vector.tensor_reduce(
                out=o_t[:, 2 + nb:3 + nb], in_=psum_s[:, nsl], axis=AX.X, op=OP.min,
            )
            # per-bank e = exp(-0.5*raw + 0.5*min_b); S_b = sum_j e (fused);
            # banks are merged on the host like shards
            nc.vector.tensor_scalar_mul(
                bias2_t[:, nb:nb + 1], o_t[:, 2 + nb:3 + nb], 0.5)
            nc.scalar.activation(
                e_t[:, nsl], psum_s[:, nsl], AF.Exp,
                bias=bias2_t[:, nb:nb + 1], scale=-0.5,
                accum_out=o_t[:, 4 + nb:5 + nb],
            )

        nc.sync.dma_start(out=out[:, :], in_=o_t[:])

    nc.finalize()
    return nc


_CACHE = {}


def _get_nc():
    if "nc" not in _CACHE:
        _CACHE["nc"] = _build()
    return _CACHE["nc"]


def _in_maps(x_mean, x_vars, y):
    m = np.ascontiguousarray(np.asarray(x_mean, dtype=np.float32))
    v = np.ascontiguousarray(np.asarray(x_vars, dtype=np.float32))
    yv = np.ascontiguousarray(np.asarray(y, dtype=np.float32))
    mT = np.ascontiguousarray(m.T)
    vT = np.ascontiguousarray(v.T)
    p = np.arange(ROWS)
    maps = []
    for c in range(NCORES):
        yTs = np.ascontiguousarray(yv[c * ROWS:(c + 1) * ROWS].T)
        consts = np.zeros((128, B + 256), np.float32)
        consts[p, c * ROWS + p] = np.float32(BIG)          # msk
        consts[p, B + p] = 1.0                             # iden
        consts[:, B + 128:B + 256] = 1.0                   # ones
        maps.append({"mT": mT, "vT": vT, "yT": yTs, "consts": consts})
    return maps


def _combine(results):
    outs = np.concatenate([results[c]["out"] for c in range(NCORES)], axis=0)
    o = outs.astype(np.float64)
    diag = -0.5 * (o[:, 0] + o[:, 1]) / BIG
    lse0 = -0.5 * o[:, 2] + np.log(o[:, 4])
    lse1 = -0.5 * o[:, 3] + np.log(o[:, 5])
    lse_nd = np.logaddexp(lse0, lse1)
    lse_f = np.logaddexp(lse_nd, diag)
    mi_lower = np.log(float(B)) + np.mean(diag - lse_f)
    mi_upper = np.mean(diag - lse_nd) + np.log(float(B - 1))
    return np.array([mi_lower, mi_upper], dtype=np.float32)


def _run(x_mean, x_vars, y, **kw):
    nc = _get_nc()
    res = bass_utils.run_bass_kernel_spmd(
        nc, _in_maps(x_mean, x_vars, y), list(range(NCORES)), **kw
    )
    return _combine(res.results), res


def kernel(x_mean, x_vars, y):
    return _run(x_mean, x_vars, y)[0]


# revision 15
# speedup vs baseline: 1.1519x; 1.0106x over previous
"""InfoNCE (CPIC) loss kernel for Trainium2, 8 NeuronCores.

Math (B=1024, D=256):
  scores[i,j] = -0.5 * sum_d( log vc[j,d] + (y[i,d]-m[j,d])^2 / vc[j,d] )
    where vc = where(v < 1e-6, v + 1e-6, v)
  mi_lower = log(B) + mean_i(diag_i - logsumexp_j scores[i,:])
  mi_upper = mean_i(diag_i - (logsumexp_{j!=i} scores[i,:] - log(B-1)))
  out = [mi_lower, mi_upper]

Decomposition on device (per core c, rows i in [128c, 128c+128)):
  raw[i,j] = sum_d y2[i,d]*r[j,d] + sum_d y[i,d]*u2[j,d] + a[j]
    r  = 1/vc, u2 = -2*m*r, a[j] = sum_d (log vc + m^2 r)
  scores = -0.5*raw.  All contractions run on the PE (K=512 accumulation
  + ones-weight matmuls that broadcast-add a[j] into every row + an
  identity-weight matmul that adds the host diag mask * 2^60).
  Per-row: extract diag (mask multiply + row-sum), add 2^60 at the diag
  (PE), min_j raw (= row max of scores, diag excluded), then
  e = exp(-0.5*raw - max) with fused per-partition sum (accum_out).
Device output per core: [128, 4] = (diag_raw*2^60, min_raw, sum_e, 0).
Host: logs/means (the cheap cross-shard reduction).

Sharding: row-shard of y; x_mean/x_vars broadcast to all 8 cores.  Host
passes transposed ([D, B]) views so no on-device transposes are needed.
"""

import os
import sys

import numpy as np

sys.path.insert(0, "/opt/trn_rl_repo")

import concourse.bass as bass  # noqa: E402,F401
import concourse.bacc as bacc  # noqa: E402
import concourse.tile as tile  # noqa: E402
from concourse.tile import add_dep_helper  # noqa: E402
import concourse.hw_specs as hw_specs  # noqa: E402
from concourse import mybir  # noqa: E402
from concourse import bass_utils  # noqa: E402
from concourse.dve_ops import (  # noqa: E402
    RECIP_APPROX_FAST_CONSTS,
    RECIPROCAL_APPROX_FAST,
)
from contextlib import ExitStack  # noqa: E402

B = 1024
D = 256
NCORES = 8
ROWS = B // NCORES  # 128
THRESHOLD = 1e-6
BIG = float(2.0**60)

F32 = mybir.dt.float32
F32R = mybir.dt.float32r
AX = mybir.AxisListType
OP = mybir.AluOpType
AF = mybir.ActivationFunctionType

# matmul operand dtype: float32r streams at 1 col/cycle (4x faster than
# float32, ~2^-13 mantissa rounding); flip env var for exact-fp32 matmuls.
MM_F32R = os.environ.get("KERNEL_MM_DTYPE", "f32r") == "f32r"
MDT = F32R if MM_F32R else F32
RECIP = os.environ.get("KERNEL_RECIP", "fast")  # fast | exact

_ACT_SET = "natural_log_exp_and_others"


def _patch_act_tables():
    """Make every activation resolve to the one set that holds ln+exp+
    square+copy, so only one ACT_TABLE_LOAD (~1.3us) is emitted.  Other
    set entries are emptied, not removed (act_func_set_id is positional)."""
    if getattr(hw_specs, "_ant_act_patch", None):
        return
    orig = hw_specs.get_activation_tables

    def patched(arch):
        tabs = orig(arch)
        if _ACT_SET not in tabs:
            return tabs
        return {k: (v if k == _ACT_SET else set()) for k, v in tabs.items()}

    hw_specs._ant_act_patch = True
    hw_specs.get_activation_tables = patched
    for mod in (bacc, bass):
        if hasattr(mod, "get_activation_tables"):
            mod.get_activation_tables = patched


def _recip(nc, out_ap, in_ap):
    if RECIP == "exact":
        nc.vector.reciprocal(out_ap, in_ap)
    else:
        c = RECIP_APPROX_FAST_CONSTS
        nc.vector._custom_dve(
            RECIPROCAL_APPROX_FAST, out=out_ap, in0=in_ap,
            s0=c["s0"], s1=c["s1"], imm2=c["imm2"],
        )


def _build():
    _patch_act_tables()
    nc = bacc.Bacc("TRN2", target_bir_lowering=False, debug=False, num_devices=8)
    mT = nc.declare_dram_parameter("mT", [D, B], F32, isOutput=False)
    vT = nc.declare_dram_parameter("vT", [D, B], F32, isOutput=False)
    yT = nc.declare_dram_parameter("yT", [D, ROWS], MDT, isOutput=False)
    consts = nc.declare_dram_parameter("consts", [128, B + 256], MDT, isOutput=False)
    out = nc.declare_dram_parameter("out", [ROWS, 6], F32, isOutput=True)

    KC = D // 128  # 2 contraction chunks

    with ExitStack() as ctx:
        tc = ctx.enter_context(tile.TileContext(nc))
        pool = ctx.enter_context(tc.tile_pool(name="main", bufs=1))
        ppool = ctx.enter_context(tc.tile_pool(name="psum", bufs=1, space="PSUM"))

        v_t = pool.tile([128, KC * B], F32, name="v")
        m_t = pool.tile([128, KC * B], F32, name="m")
        y_t = pool.tile([128, KC * ROWS], MDT, name="y")
        y2_t = pool.tile([128, KC * ROWS], MDT, name="y2")
        vc_t = [pool.tile([128, B], F32, name=f"vc{k}") for k in range(KC)]
        m2_t = [pool.tile([128, B], F32, name=f"m2{k}") for k in range(KC)]
        tm_t = [pool.tile([128, B], F32, name=f"tm{k}") for k in range(KC)]
        r_t = [pool.tile([128, B], MDT, name=f"r{k}") for k in range(KC)]
        u2_t = [pool.tile([128, B], MDT, name=f"u2{k}") for k in range(KC)]
        mu_t = [pool.tile([128, B], MDT, name=f"mu{k}") for k in range(KC)]
        lv_t = [pool.tile([128, B], MDT, name=f"lv{k}") for k in range(KC)]
        consts_t = pool.tile([128, B + 256], MDT, name="consts")
        msk_t = consts_t[:, 0:B]
        iden_t = consts_t[:, B:B + 128]
        ones_t = consts_t[:, B + 128:B + 256]
        scr_t = pool.tile([ROWS, B], F32, name="scr")
        e_t = pool.tile([ROWS, B], F32, name="e")
        o_t = pool.tile([ROWS, 6], F32, name="o")
        bias2_t = pool.tile([ROWS, 2], F32, name="bias2")
        bias_t = pool.tile([ROWS, 1], F32, name="bias")

        psum_s = ppool.tile([ROWS, B], F32, name="scores")

        vT3 = vT.rearrange("(c p) b -> p c b", p=128)
        mT3 = mT.rearrange("(c p) b -> p c b", p=128)
        yT3 = yT.rearrange("(c p) i -> p c i", p=128)

        nc.sync.dma_start(out=v_t[:, 0:B], in_=vT3[:, 0, :])
        nc.scalar.dma_start(out=m_t[:, 0:B], in_=mT3[:, 0, :])
        nc.sync.dma_start(out=v_t[:, B:], in_=vT3[:, 1, :])
        nc.scalar.dma_start(out=m_t[:, B:], in_=mT3[:, 1, :])
        nc.sync.dma_start(out=y_t[:].rearrange("p (c i) -> p c i", c=KC), in_=yT3)
        nc.scalar.dma_start(out=consts_t[:], in_=consts[:, :])

        prev_last = None
        with nc.allow_low_precision(reason="f32r matmul operands"):
            nc.scalar.activation(y2_t[:], y_t[:], AF.Square)
            for k in range(KC):
                vk = v_t[:, k * B:(k + 1) * B]
                mk = m_t[:, k * B:(k + 1) * B]
                # vc = v + T*(v < T)
                i_ts = nc.vector.tensor_scalar(
                    out=tm_t[k][:], in0=vk, scalar1=float(THRESHOLD),
                    scalar2=float(THRESHOLD), op0=OP.is_lt, op1=OP.mult,
                )
                if prev_last is not None:
                    # keep the DVE stream chunk-major: chunk k+1 must not
                    # sit ahead of chunk k's chain (head-of-line on DMA wait)
                    add_dep_helper(i_ts.ins, prev_last.ins, sync=False,
                                   reason="chunk order")
                nc.vector.tensor_add(vc_t[k][:], tm_t[k][:], vk)
                _recip(nc, r_t[k][:], vc_t[k][:])
                # u2 = -2*m*r ; mu = m^2*r = (m*-0.5)*u2
                nc.vector.scalar_tensor_tensor(
                    out=u2_t[k][:], in0=mk, scalar=-2.0, in1=r_t[k][:],
                    op0=OP.mult, op1=OP.mult,
                )
                nc.scalar.activation(m2_t[k][:], mk, AF.Square)
                prev_last = nc.vector.scalar_tensor_tensor(
                    out=mu_t[k][:], in0=m2_t[k][:], scalar=1.0, in1=r_t[k][:],
                    op0=OP.bypass, op1=OP.mult,
                )
                nc.scalar.activation(lv_t[k][:], vc_t[k][:], AF.Ln)

        # raw = y2.T@r + y.T@u2 + ones.T@(lv+mu)   (per 512-col PSUM bank),
        # then per-bank: diag partial (mask multiply+reduce), diag mask add
        # on the PE (I.T @ msk, msk holds 2^60 at diag), partial row min.
        dtmp = pool.tile([ROWS, 2], F32, name="dtmp")
        mtmp = pool.tile([ROWS, 2], F32, name="mtmp")
        NB = B // 512
        for nb in range(NB):
            nsl = slice(nb * 512, (nb + 1) * 512)
            seq = []
            for k in range(KC):
                ksl = slice(k * ROWS, (k + 1) * ROWS)
                seq.append((y2_t[:, ksl], r_t[k][:, nsl]))
                seq.append((y_t[:, ksl], u2_t[k][:, nsl]))
            for k in range(KC):
                seq.append((ones_t[:], lv_t[k][:, nsl]))
                seq.append((ones_t[:], mu_t[k][:, nsl]))
            for si, (lhsT, rhs) in enumerate(seq):
                nc.tensor.matmul(
                    psum_s[:, nsl], lhsT, rhs,
                    start=(si == 0), stop=(si == len(seq) - 1),
                )
        for nb in range(NB):
            nsl = slice(nb * 512, (nb + 1) * 512)
            nc.vector.tensor_mul(scr_t[:, nsl], psum_s[:, nsl], msk_t[:, nsl])
            nc.vector.tensor_reduce(
                out=o_t[:, nb:nb + 1], in_=scr_t[:, nsl], axis=AX.X, op=OP.add,
            )
            nc.tensor.matmul(
                psum_s[:, nsl], iden_t[:], msk_t[:, nsl],
                start=False, stop=True, skip_group_check=True,
            )
            nc.vector.tensor_reduce(
                out=o_t[:, 2 + nb:3 + nb], in_=psum_s[:, nsl], axis=AX.X, op=OP.min,
            )
            # per-bank e = exp(-0.5*raw + 0.5*min_b); S_b = sum_j e (fused);
            # banks are merged on the host like shards
            nc.vector.tensor_scalar_mul(
                bias2_t[:, nb:nb + 1], o_t[:, 2 + nb:3 + nb], 0.5)
            nc.scalar.activation(
                e_t[:, nsl], psum_s[:, nsl], AF.Exp,
                bias=bias2_t[:, nb:nb + 1], scale=-0.5,
                accum_out=o_t[:, 4 + nb:5 + nb],
            )

        nc.sync.dma_start(out=out[:, :], in_=o_t[:])

    nc.finalize()
    return nc


_CACHE = {}


def _get_nc():
    if "nc" not in _CACHE:
        _CACHE["nc"] = _build()
    return _CACHE["nc"]


def _in_maps(x_mean, x_vars, y):
    m = np.ascontiguousarray(np.asarray(x_mean, dtype=np.float32))
    v = np.ascontiguousarray(np.asarray(x_vars, dtype=np.float32))
    yv = np.ascontiguousarray(np.asarray(y, dtype=np.float32))
    mT = np.ascontiguousarray(m.T)
    vT = np.ascontiguousarray(v.T)
    p = np.arange(ROWS)
    maps = []
    for c in range(NCORES):
        yTs = np.ascontiguousarray(yv[c * ROWS:(c + 1) * ROWS].T)
        consts = np.zeros((128, B + 256), np.float32)
        consts[p, c * ROWS + p] = np.float32(BIG)          # msk
        consts[p, B + p] = 1.0                             # iden
        consts[:, B + 128:B + 256] = 1.0                   # ones
        maps.append({"mT": mT, "vT": vT, "yT": yTs, "consts": consts})
    return maps


def _combine(results):
    outs = np.concatenate([results[c]["out"] for c in range(NCORES)], axis=0)
    o = outs.astype(np.float64)
    diag = -0.5 * (o[:, 0] + o[:, 1]) / BIG
    lse0 = -0.5 * o[:, 2] + np.log(o[:, 4])
    lse1 = -0.5 * o[:, 3] + np.log(o[:, 5])
    lse_nd = np.logaddexp(lse0, lse1)
    lse_f = np.logaddexp(lse_nd, diag)
    mi_lower = np.log(float(B)) + np.mean(diag - lse_f)
    mi_upper = np.mean(diag - lse_nd) + np.log(float(B - 1))
    return np.array([mi_lower, mi_upper], dtype=np.float32)


def _run(x_mean, x_vars, y, **kw):
    nc = _get_nc()
    res = bass_utils.run_bass_kernel_spmd(
        nc, _in_maps(x_mean, x_vars, y), list(range(NCORES)), **kw
    )
    return _combine(res.results), res


def kernel(x_mean, x_vars, y):
    return _run(x_mean, x_vars, y)[0]
